# revision 1
# baseline (speedup 1.0000x reference)
"""Multi-head attention (B=4, S=2048, E=1024, H=16) on 8 trn2 NeuronCores.

Sharding: data-parallel over B (4) x tensor-parallel over H (2 halves of 8
heads). Core c handles batch c//2, head-half c%2. Column-parallel qkv_proj,
row-parallel out_proj; the all-reduce of the two partial outputs per batch is
done on the host during unshard (a sum of two arrays), as is the final
transpose (the device emits out^T to keep DMA writes contiguous).

Device kernel (identical program on all 8 cores, fp32r matmuls):
  phase 1/2: qk^T = Wqk_loc @ x^T  [1024, 2048] and v = x @ Wv_loc^T + bv
             [2048, 512] (bias via a K=1 ones-row matmul)
  phase 3:   per head pair, flash-style over 128-key tiles:
             scores^T pairs row-packed at partitions 0/64, ACT exp with the
             1/sqrt(E) scale folded in, PV matmul with stationary [v | 1]
             (even head, M=65: ctx at partitions 0-63, sums at 64) or
             [1 | 0*63 | v] (odd head, M=128: sums at 0, ctx at 64-127) so
             the softmax denominator rides along for free; normalization by
             reciprocal of a DRAM-bounce partition-broadcast of the sums row
  phase 4:   out^T partial = Wout_loc^T-stationary matmuls + bias (bout on
             even cores only, zeros on odd, so the host sum adds it once)
"""
import sys

import numpy as np

sys.path.insert(0, "/opt/trn_rl_repo")

import concourse.bacc as bacc
import concourse.mybir as mybir
import concourse.tile as tile
from concourse.bass_utils import run_bass_kernel_spmd
from concourse.tile_rust import add_dep_helper

F32 = mybir.dt.float32
F32R = mybir.dt.float32r
EXP = mybir.ActivationFunctionType.Exp

B, S, E, H, HD = 4, 2048, 1024, 16, 64
HL = 8            # heads per core
SCALE = 1.0 / np.sqrt(E).astype(np.float32)

# cons layout: [0:128] ones, [128:640] bv, [640:1412] v1 pad pattern
ONES_OFF, BV_OFF, VPAD_OFF, CONS_LEN = 0, 128, 640, 1412
V1W = 4 * 65 + 4 * 128   # 772 cols per key-tile block in v1


def build_nc():
    nc = bacc.Bacc("TRN2", target_bir_lowering=False, debug=False, num_devices=8)
    xw_d = nc.declare_dram_parameter("xw", [E, 3584], F32, isOutput=False)
    bqk_d = nc.declare_dram_parameter("bqk", [E, 1], F32, isOutput=False)
    cons_d = nc.declare_dram_parameter("cons", [1, CONS_LEN], F32, isOutput=False)
    wo_d = nc.declare_dram_parameter("wo", [512, E], F32, isOutput=False)
    bout_d = nc.declare_dram_parameter("bout", [E, 1], F32, isOutput=False)
    out_d = nc.declare_dram_parameter("outT", [E, S], F32, isOutput=True)
    rb = nc.dram_tensor("rb", [8, S], F32)   # sums bounce rows

    with tile.TileContext(nc) as tc:
      with tc.tile_pool(name="pp", bufs=1) as pp:
        bqk_s = pp.tile([128, 8, 1], F32)
        bout_s = pp.tile([128, 8, 1], F32)
        cons_s = pp.tile([1, CONS_LEN], F32R)
        nc.sync.dma_start(out=bqk_s, in_=bqk_d[:, :].rearrange("(m p) o -> p m o", p=128))
        nc.sync.dma_start(out=bout_s, in_=bout_d[:, :].rearrange("(m p) o -> p m o", p=128))
        nc.sync.dma_start(out=cons_s, in_=cons_d[:, :].bitcast(F32R))

        with tc.tile_pool(name="pa", bufs=1) as pa:
            qk_s = pa.tile([128, 8, S], F32R)     # qk^T: m-tile 0-3 q, 4-7 k
            v1_s = pa.tile([128, 16, V1W], F32R)  # per key-tile [v|1]x4, [1|0|v]x4

            # v1 pad pattern (ones + zero pads; v cols overwritten by evicts)
            for jt in range(16):
                nc.sync.dma_start(
                    out=v1_s[:, jt, :],
                    in_=cons_d[0:1, VPAD_OFF:VPAD_OFF + V1W].bitcast(F32R)
                    .to_broadcast([128, V1W]))

            # ---- phases 1+2: qk^T and v
            with tc.tile_pool(name="p12", bufs=1) as p12, \
                 tc.tile_pool(name="ps12", bufs=1, space="PSUM") as ps12:
                wv_s = p12.tile([128, 8, 512], F32R)
                nc.sync.dma_start(
                    out=wv_s,
                    in_=xw_d[:, 3072:3584].bitcast(F32R)
                    .rearrange("(kt p) n -> p kt n", p=128))
                for mh in range(2):
                    wqk_s = p12.tile([128, 8, 512], F32R, tag="wqk", bufs=2)
                    nc.sync.dma_start(
                        out=wqk_s,
                        in_=xw_d[:, mh * 512:(mh + 1) * 512].bitcast(F32R)
                        .rearrange("(kt p) n -> p kt n", p=128))
                    for ic in range(4):
                        xc_s = p12.tile([128, 8, 512], F32R, tag="xc", bufs=2)
                        nc.sync.dma_start(
                            out=xc_s,
                            in_=xw_d[:, 1024 + ic * 512:1024 + (ic + 1) * 512]
                            .bitcast(F32R).rearrange("(kt p) n -> p kt n", p=128))
                        for m in range(4):
                            pq = ps12.tile([128, 512], F32, tag="pq", bufs=3)
                            for kt in range(8):
                                nc.tensor.matmul(
                                    out=pq, lhsT=wqk_s[:, kt, m * 128:(m + 1) * 128],
                                    rhs=xc_s[:, kt, :],
                                    start=(kt == 0), stop=(kt == 7))
                            nc.vector.tensor_scalar_add(
                                qk_s[:, mh * 4 + m, ic * 512:(ic + 1) * 512],
                                pq, bqk_s[:, mh * 4 + m, 0:1])
                        if mh == 0:
                            for st in range(4):
                                jt = ic * 4 + st
                                pv = ps12.tile([128, 512], F32, tag="pv", bufs=2)
                                for kt in range(8):
                                    nc.tensor.matmul(
                                        out=pv,
                                        lhsT=xc_s[:, kt, st * 128:(st + 1) * 128],
                                        rhs=wv_s[:, kt, :],
                                        start=(kt == 0), stop=False)
                                nc.tensor.matmul(
                                    out=pv, lhsT=cons_s[0:1, ONES_OFF:ONES_OFF + 128],
                                    rhs=cons_s[0:1, BV_OFF:BV_OFF + 512],
                                    start=False, stop=True)
                                # evict: even heads -> [v|1] blocks, odd -> [1|0|v]
                                nc.vector.tensor_copy(
                                    v1_s[:, jt, 0:260]
                                    .rearrange("p (b c) -> p b c", c=65)[:, :, 0:64],
                                    pv[:, :].rearrange("p (b t d) -> p b t d", t=2, d=64)
                                    [:, :, 0, :])
                                nc.vector.tensor_copy(
                                    v1_s[:, jt, 260:V1W]
                                    .rearrange("p (b c) -> p b c", c=128)[:, :, 64:128],
                                    pv[:, :].rearrange("p (b t d) -> p b t d", t=2, d=64)
                                    [:, :, 1, :])

            # ---- phase 3: attention per head pair
            with tc.tile_pool(name="pc", bufs=1) as pc:
                ctx_s = pc.tile([128, 4, S], F32R)
                with tc.tile_pool(name="p3", bufs=1) as p3, \
                     tc.tile_pool(name="ps3", bufs=1, space="PSUM") as ps3:
                    for p in range(4):
                        for icp in range(2):
                            s_e = ps3.tile([128, 1024], F32, tag="s_e", bufs=1)
                            s_o = ps3.tile([128, 1024], F32, tag="s_o", bufs=1)
                            pv_e = ps3.tile([65, 1024], F32, tag="pv_e", bufs=1)
                            pv_o = ps3.tile([128, 1024], F32, tag="pv_o", bufs=1)
                            for jt in range(16):
                                for ih in range(2):
                                    icol = icp * 1024 + ih * 512
                                    nc.tensor.matmul(
                                        out=s_e[:, ih * 512:(ih + 1) * 512],
                                        lhsT=qk_s[0:64, 4 + p, jt * 128:(jt + 1) * 128],
                                        rhs=qk_s[0:64, p, icol:icol + 512],
                                        start=True, stop=True)
                                    nc.tensor.matmul(
                                        out=s_o[:, ih * 512:(ih + 1) * 512],
                                        lhsT=qk_s[64:128, 4 + p, jt * 128:(jt + 1) * 128],
                                        rhs=qk_s[64:128, p, icol:icol + 512],
                                        start=True, stop=True)
                                e_e = p3.tile([128, 1024], F32R, tag="e", bufs=3)
                                nc.scalar.activation(out=e_e, in_=s_e, func=EXP,
                                                     scale=float(SCALE))
                                e_o = p3.tile([128, 1024], F32R, tag="e", bufs=3)
                                nc.scalar.activation(out=e_o, in_=s_o, func=EXP,
                                                     scale=float(SCALE))
                                for ih in range(2):
                                    sl = slice(ih * 512, (ih + 1) * 512)
                                    nc.tensor.matmul(
                                        out=pv_e[:, sl],
                                        lhsT=v1_s[:, jt, p * 65:p * 65 + 65],
                                        rhs=e_e[:, sl],
                                        start=(jt == 0), stop=(jt == 15))
                                    nc.tensor.matmul(
                                        out=pv_o[:, sl],
                                        lhsT=v1_s[:, jt, 260 + p * 128:260 + (p + 1) * 128],
                                        rhs=e_o[:, sl],
                                        start=(jt == 0), stop=(jt == 15))
                            # evict pv to sbuf (frees psum), then normalize
                            pvt_e = p3.tile([65, 1024], F32, tag="pvt_e", bufs=2)
                            pvt_o = p3.tile([128, 1024], F32, tag="pvt_o", bufs=2)
                            nc.vector.tensor_copy(pvt_e, pv_e)
                            nc.vector.tensor_copy(pvt_o, pv_o)
                            ic_sl = slice(icp * 1024, (icp + 1) * 1024)
                            st_e = nc.sync.dma_start(out=rb[2 * p:2 * p + 1, ic_sl],
                                                     in_=pvt_e[64:65, :])
                            st_o = nc.sync.dma_start(out=rb[2 * p + 1:2 * p + 2, ic_sl],
                                                     in_=pvt_o[0:1, :])
                            rep = p3.tile([128, 1024], F32, tag="rep", bufs=1)
                            ld_e = nc.gpsimd.dma_start(
                                out=rep[0:64, :],
                                in_=rb[2 * p:2 * p + 1, ic_sl].to_broadcast([64, 1024]))
                            ld_o = nc.gpsimd.dma_start(
                                out=rep[64:128, :],
                                in_=rb[2 * p + 1:2 * p + 2, ic_sl].to_broadcast([64, 1024]))
                            add_dep_helper(ld_e.ins, st_e.ins, sync=True, reason="raw_e")
                            add_dep_helper(ld_o.ins, st_o.ins, sync=True, reason="raw_o")
                            rrec = p3.tile([128, 1024], F32, tag="rrec", bufs=2)
                            rscr = p3.tile([128, 1024], F32, tag="rscr", bufs=1)
                            nc.vector.reciprocal_approx_accurate(
                                out=rrec, in_=rep, scratch=rscr)
                            nc.vector.tensor_mul(ctx_s[0:64, p, ic_sl],
                                                 pvt_e[0:64, :], rrec[0:64, :])
                            nc.vector.tensor_mul(ctx_s[64:128, p, ic_sl],
                                                 pvt_o[64:128, :], rrec[64:128, :])

                # ---- phase 4: out projection (partial), written as out^T
                with tc.tile_pool(name="p4", bufs=1) as p4, \
                     tc.tile_pool(name="ps4", bufs=1, space="PSUM") as ps4:
                    wo_s = p4.tile([128, 4, E], F32R)
                    nc.sync.dma_start(
                        out=wo_s,
                        in_=wo_d[:, :].bitcast(F32R).rearrange("(ct p) e -> p ct e", p=128))
                    for et in range(8):
                        for i4 in range(4):
                            po = ps4.tile([128, 512], F32, tag="po", bufs=4)
                            for ct in range(4):
                                nc.tensor.matmul(
                                    out=po, lhsT=wo_s[:, ct, et * 128:(et + 1) * 128],
                                    rhs=ctx_s[:, ct, i4 * 512:(i4 + 1) * 512],
                                    start=(ct == 0), stop=(ct == 3))
                            ot = p4.tile([128, 512], F32, tag="ot", bufs=4)
                            nc.vector.tensor_scalar_add(ot, po, bout_s[:, et, 0:1])
                            nc.sync.dma_start(
                                out=out_d[et * 128:(et + 1) * 128,
                                          i4 * 512:(i4 + 1) * 512],
                                in_=ot)
    nc.compile()
    return nc


_NC = None


def _get_nc():
    global _NC
    if _NC is None:
        _NC = build_nc()
    return _NC


def make_in_maps(query, Wqkv, bqkv, Wout, bout):
    query = np.asarray(query, dtype=np.float32)
    Wqkv = np.asarray(Wqkv, dtype=np.float32)
    bqkv = np.asarray(bqkv, dtype=np.float32)
    Wout = np.asarray(Wout, dtype=np.float32)
    bout = np.asarray(bout, dtype=np.float32)

    in_maps = []
    for c in range(8):
        b, hh = c // 2, c % 2
        heads = np.arange(hh * HL, hh * HL + HL)
        dims = (heads[:, None] * HD + np.arange(HD)[None, :]).reshape(-1)  # [512]
        q_rows, k_rows, v_rows = dims, E + dims, 2 * E + dims

        xw = np.empty((E, 3584), np.float32)
        xw[:, 0:512] = Wqkv[q_rows].T
        xw[:, 512:1024] = Wqkv[k_rows].T
        xw[:, 1024:3072] = query[b].T
        xw[:, 3072:3584] = Wqkv[v_rows].T

        bqk = np.concatenate([bqkv[q_rows], bqkv[k_rows]]).reshape(E, 1)

        cons = np.zeros((1, CONS_LEN), np.float32)
        cons[0, ONES_OFF:ONES_OFF + 128] = 1.0
        cons[0, BV_OFF:BV_OFF + 512] = bqkv[v_rows]
        vpad = np.zeros(V1W, np.float32)
        for i in range(4):
            vpad[i * 65 + 64] = 1.0          # even-head ones col
            vpad[260 + i * 128] = 1.0        # odd-head ones col
        cons[0, VPAD_OFF:VPAD_OFF + V1W] = vpad

        wo = np.ascontiguousarray(Wout[:, dims].T)          # [512, E]
        bo = (bout if hh == 0 else np.zeros_like(bout)).reshape(E, 1)

        in_maps.append({
            "xw": xw, "bqk": np.ascontiguousarray(bqk),
            "cons": cons, "wo": wo, "bout": np.ascontiguousarray(bo),
        })
    return in_maps


def gather(results):
    out = np.empty((B, S, E), np.float32)
    for b in range(B):
        acc = results[2 * b]["outT"] + results[2 * b + 1]["outT"]   # [E, S]
        out[b] = acc.T
    return out


def kernel(query, key, value, Wqkv, bqkv, Wout, bout):
    # key/value are unused by the reference module (qkv all from query)
    nc = _get_nc()
    in_maps = make_in_maps(query, Wqkv, bqkv, Wout, bout)
    res = run_bass_kernel_spmd(nc, in_maps, list(range(8)))
    return gather(res.results)


# revision 3
# speedup vs baseline: 252.4880x; 252.4880x over previous
"""Multi-head attention (B=4, S=2048, E=1024, H=16) on 8 trn2 NeuronCores.

Sharding: data-parallel over B (4) x tensor-parallel over H (2 halves of 8
heads). Core c handles batch c//2, head-half c%2. Column-parallel qkv_proj,
row-parallel out_proj; the all-reduce of the two partial outputs per batch is
done on the host during unshard (a sum of two arrays), as is the final
transpose (the device emits out^T to keep DMA writes contiguous).

Device kernel (identical program on all 8 cores, fp32r matmuls):
  phase 1/2: qk^T = Wqk_loc @ x^T  [1024, 2048] and v = x @ Wv_loc^T + bv
             [2048, 512] (bias via a K=1 ones-row matmul)
  phase 3:   per head pair, flash-style over 128-key tiles:
             scores^T pairs row-packed at partitions 0/64, ACT exp with the
             1/sqrt(E) scale folded in, PV matmul with stationary [v | 1]
             (even head, M=65: ctx at partitions 0-63, sums at 64) or
             [1 | 0*63 | v] (odd head, M=128: sums at 0, ctx at 64-127) so
             the softmax denominator rides along for free; normalization by
             reciprocal of a DRAM-bounce partition-broadcast of the sums row
  phase 4:   out^T partial = Wout_loc^T-stationary matmuls + bias (bout on
             even cores only, zeros on odd, so the host sum adds it once)
"""
import sys

import numpy as np

sys.path.insert(0, "/opt/trn_rl_repo")

import concourse.bacc as bacc
import concourse.mybir as mybir
import concourse.tile as tile
from concourse.bass_utils import run_bass_kernel_spmd
from concourse.tile_rust import add_dep_helper

F32 = mybir.dt.float32
F32R = mybir.dt.float32r
EXP = mybir.ActivationFunctionType.Exp

B, S, E, H, HD = 4, 2048, 1024, 16, 64
HL = 8            # heads per core
SCALE = 1.0 / np.sqrt(E).astype(np.float32)

# cons layout: [0:128] ones, [128:640] bv, [640:1412] v1 pad pattern
ONES_OFF, BV_OFF, VPAD_OFF, CONS_LEN = 0, 128, 640, 1412
V1W = 4 * 65 + 4 * 128   # 772 cols per key-tile block in v1


def build_nc():
    nc = bacc.Bacc("TRN2", target_bir_lowering=False, debug=False, num_devices=8)
    xw_d = nc.declare_dram_parameter("xw", [E, 3584], F32, isOutput=False)
    bqk_d = nc.declare_dram_parameter("bqk", [E, 1], F32, isOutput=False)
    cons_d = nc.declare_dram_parameter("cons", [1, CONS_LEN], F32, isOutput=False)
    wo_d = nc.declare_dram_parameter("wo", [512, E], F32, isOutput=False)
    bout_d = nc.declare_dram_parameter("bout", [E, 1], F32, isOutput=False)
    out_d = nc.declare_dram_parameter("outT", [E, S], F32, isOutput=True)
    rb = nc.dram_tensor("rb", [8, S], F32)   # sums bounce rows

    with tile.TileContext(nc) as tc:
      with tc.tile_pool(name="pp", bufs=1) as pp:
        bqk_s = pp.tile([128, 8, 1], F32)
        bout_s = pp.tile([128, 8, 1], F32)
        cons_s = pp.tile([1, CONS_LEN], F32R)
        nc.gpsimd.dma_start(out=bqk_s, in_=bqk_d[:, :].rearrange("(m p) o -> p m o", p=128))
        nc.gpsimd.dma_start(out=bout_s, in_=bout_d[:, :].rearrange("(m p) o -> p m o", p=128))
        nc.gpsimd.dma_start(out=cons_s, in_=cons_d[:, :].bitcast(F32R))

        with tc.tile_pool(name="pa", bufs=1) as pa:
            qk_s = pa.tile([128, 8, S], F32R)     # qk^T: m-tile 0-3 q, 4-7 k
            v1_s = pa.tile([128, 16, V1W], F32R)  # per key-tile [v|1]x4, [1|0|v]x4

            # v1 pad pattern (ones + zero pads; v cols overwritten by evicts)
            for jt in range(16):
                nc.gpsimd.dma_start(
                    out=v1_s[:, jt, :],
                    in_=cons_d[0:1, VPAD_OFF:VPAD_OFF + V1W].bitcast(F32R)
                    .to_broadcast([128, V1W]))

            # ---- phases 1+2: qk^T and v
            with tc.tile_pool(name="p12", bufs=1) as p12, \
                 tc.tile_pool(name="ps12", bufs=1, space="PSUM") as ps12:
                wv_s = p12.tile([128, 8, 512], F32R)
                for mh in range(2):
                    wqk_s = p12.tile([128, 8, 512], F32R, tag="wqk", bufs=2)
                    for kt in range(8):
                        nc.sync.dma_start(
                            out=wqk_s[:, kt, :],
                            in_=xw_d[kt * 128:(kt + 1) * 128,
                                     mh * 512:(mh + 1) * 512].bitcast(F32R))
                    if mh == 0:
                        for kt in range(8):
                            nc.sync.dma_start(
                                out=wv_s[:, kt, :],
                                in_=xw_d[kt * 128:(kt + 1) * 128, 3072:3584]
                                .bitcast(F32R))
                    for ic in range(4):
                        xc_s = p12.tile([128, 8, 512], F32R, tag="xc", bufs=2)
                        for kt in range(8):
                            nc.sync.dma_start(
                                out=xc_s[:, kt, :],
                                in_=xw_d[kt * 128:(kt + 1) * 128,
                                         1024 + ic * 512:1024 + (ic + 1) * 512]
                                .bitcast(F32R))
                        for m in range(4):
                            pq = ps12.tile([128, 512], F32, tag="pq", bufs=3)
                            for kt in range(8):
                                nc.tensor.matmul(
                                    out=pq, lhsT=wqk_s[:, kt, m * 128:(m + 1) * 128],
                                    rhs=xc_s[:, kt, :],
                                    start=(kt == 0), stop=(kt == 7))
                            nc.vector.tensor_scalar_add(
                                qk_s[:, mh * 4 + m, ic * 512:(ic + 1) * 512],
                                pq, bqk_s[:, mh * 4 + m, 0:1])
                        if mh == 0:
                            for st in range(4):
                                jt = ic * 4 + st
                                pv = ps12.tile([128, 512], F32, tag="pv", bufs=2)
                                for kt in range(8):
                                    nc.tensor.matmul(
                                        out=pv,
                                        lhsT=xc_s[:, kt, st * 128:(st + 1) * 128],
                                        rhs=wv_s[:, kt, :],
                                        start=(kt == 0), stop=False)
                                nc.tensor.matmul(
                                    out=pv, lhsT=cons_s[0:1, ONES_OFF:ONES_OFF + 128],
                                    rhs=cons_s[0:1, BV_OFF:BV_OFF + 512],
                                    start=False, stop=True)
                                # evict: even heads -> [v|1] blocks, odd -> [1|0|v]
                                nc.vector.tensor_copy(
                                    v1_s[:, jt, 0:260]
                                    .rearrange("p (b c) -> p b c", c=65)[:, :, 0:64],
                                    pv[:, :].rearrange("p (b t d) -> p b t d", t=2, d=64)
                                    [:, :, 0, :])
                                nc.vector.tensor_copy(
                                    v1_s[:, jt, 260:V1W]
                                    .rearrange("p (b c) -> p b c", c=128)[:, :, 64:128],
                                    pv[:, :].rearrange("p (b t d) -> p b t d", t=2, d=64)
                                    [:, :, 1, :])

            # ---- phase 3: attention per head pair
            with tc.tile_pool(name="pc", bufs=1) as pc:
                ctx_s = pc.tile([128, 4, S], F32R)
                with tc.tile_pool(name="p3", bufs=1) as p3, \
                     tc.tile_pool(name="ps3", bufs=1, space="PSUM") as ps3:
                    for p in range(4):
                        for icp in range(2):
                            s_e = ps3.tile([128, 1024], F32, tag="s_e", bufs=1)
                            s_o = ps3.tile([128, 1024], F32, tag="s_o", bufs=1)
                            pv_e = ps3.tile([65, 1024], F32, tag="pv_e", bufs=1)
                            pv_o = ps3.tile([128, 1024], F32, tag="pv_o", bufs=1)
                            for jt in range(16):
                                for ih in range(2):
                                    icol = icp * 1024 + ih * 512
                                    nc.tensor.matmul(
                                        out=s_e[:, ih * 512:(ih + 1) * 512],
                                        lhsT=qk_s[0:64, 4 + p, jt * 128:(jt + 1) * 128],
                                        rhs=qk_s[0:64, p, icol:icol + 512],
                                        start=True, stop=True)
                                    nc.tensor.matmul(
                                        out=s_o[:, ih * 512:(ih + 1) * 512],
                                        lhsT=qk_s[64:128, 4 + p, jt * 128:(jt + 1) * 128],
                                        rhs=qk_s[64:128, p, icol:icol + 512],
                                        start=True, stop=True)
                                e_e = p3.tile([128, 1024], F32R, tag="e", bufs=3)
                                nc.scalar.activation(out=e_e, in_=s_e, func=EXP,
                                                     scale=float(SCALE))
                                e_o = p3.tile([128, 1024], F32R, tag="e", bufs=3)
                                nc.scalar.activation(out=e_o, in_=s_o, func=EXP,
                                                     scale=float(SCALE))
                                for ih in range(2):
                                    sl = slice(ih * 512, (ih + 1) * 512)
                                    nc.tensor.matmul(
                                        out=pv_e[:, sl],
                                        lhsT=v1_s[:, jt, p * 65:p * 65 + 65],
                                        rhs=e_e[:, sl],
                                        start=(jt == 0), stop=(jt == 15))
                                    nc.tensor.matmul(
                                        out=pv_o[:, sl],
                                        lhsT=v1_s[:, jt, 260 + p * 128:260 + (p + 1) * 128],
                                        rhs=e_o[:, sl],
                                        start=(jt == 0), stop=(jt == 15))
                            # evict pv to sbuf (frees psum), then normalize
                            pvt_e = p3.tile([65, 1024], F32, tag="pvt_e", bufs=2)
                            pvt_o = p3.tile([128, 1024], F32, tag="pvt_o", bufs=2)
                            nc.vector.tensor_copy(pvt_e, pv_e)
                            nc.vector.tensor_copy(pvt_o, pv_o)
                            ic_sl = slice(icp * 1024, (icp + 1) * 1024)
                            st_e = nc.sync.dma_start(out=rb[2 * p:2 * p + 1, ic_sl],
                                                     in_=pvt_e[64:65, :])
                            st_o = nc.sync.dma_start(out=rb[2 * p + 1:2 * p + 2, ic_sl],
                                                     in_=pvt_o[0:1, :])
                            rep = p3.tile([128, 1024], F32, tag="rep", bufs=1)
                            ld_e = nc.gpsimd.dma_start(
                                out=rep[0:64, :],
                                in_=rb[2 * p:2 * p + 1, ic_sl].to_broadcast([64, 1024]))
                            ld_o = nc.gpsimd.dma_start(
                                out=rep[64:128, :],
                                in_=rb[2 * p + 1:2 * p + 2, ic_sl].to_broadcast([64, 1024]))
                            add_dep_helper(ld_e.ins, st_e.ins, sync=True, reason="raw_e")
                            add_dep_helper(ld_o.ins, st_o.ins, sync=True, reason="raw_o")
                            rrec = p3.tile([128, 1024], F32, tag="rrec", bufs=2)
                            rscr = p3.tile([128, 1024], F32, tag="rscr", bufs=1)
                            nc.vector.reciprocal_approx_accurate(
                                out=rrec, in_=rep, scratch=rscr)
                            nc.vector.tensor_mul(ctx_s[0:64, p, ic_sl],
                                                 pvt_e[0:64, :], rrec[0:64, :])
                            nc.vector.tensor_mul(ctx_s[64:128, p, ic_sl],
                                                 pvt_o[64:128, :], rrec[64:128, :])

                # ---- phase 4: out projection (partial), written as out^T
                with tc.tile_pool(name="p4", bufs=1) as p4, \
                     tc.tile_pool(name="ps4", bufs=1, space="PSUM") as ps4:
                    wo_s = p4.tile([128, 4, E], F32R)
                    for ct in range(4):
                        nc.sync.dma_start(
                            out=wo_s[:, ct, :],
                            in_=wo_d[ct * 128:(ct + 1) * 128, :].bitcast(F32R))
                    for et in range(8):
                        for i4 in range(4):
                            po = ps4.tile([128, 512], F32, tag="po", bufs=4)
                            for ct in range(4):
                                nc.tensor.matmul(
                                    out=po, lhsT=wo_s[:, ct, et * 128:(et + 1) * 128],
                                    rhs=ctx_s[:, ct, i4 * 512:(i4 + 1) * 512],
                                    start=(ct == 0), stop=(ct == 3))
                            ot = p4.tile([128, 512], F32, tag="ot", bufs=4)
                            nc.vector.tensor_scalar_add(ot, po, bout_s[:, et, 0:1])
                            nc.sync.dma_start(
                                out=out_d[et * 128:(et + 1) * 128,
                                          i4 * 512:(i4 + 1) * 512],
                                in_=ot)
    nc.compile()
    return nc


_NC = None


def _get_nc():
    global _NC
    if _NC is None:
        _NC = build_nc()
    return _NC


def make_in_maps(query, Wqkv, bqkv, Wout, bout):
    query = np.asarray(query, dtype=np.float32)
    Wqkv = np.asarray(Wqkv, dtype=np.float32)
    bqkv = np.asarray(bqkv, dtype=np.float32)
    Wout = np.asarray(Wout, dtype=np.float32)
    bout = np.asarray(bout, dtype=np.float32)

    in_maps = []
    for c in range(8):
        b, hh = c // 2, c % 2
        heads = np.arange(hh * HL, hh * HL + HL)
        dims = (heads[:, None] * HD + np.arange(HD)[None, :]).reshape(-1)  # [512]
        q_rows, k_rows, v_rows = dims, E + dims, 2 * E + dims

        xw = np.empty((E, 3584), np.float32)
        xw[:, 0:512] = Wqkv[q_rows].T
        xw[:, 512:1024] = Wqkv[k_rows].T
        xw[:, 1024:3072] = query[b].T
        xw[:, 3072:3584] = Wqkv[v_rows].T

        bqk = np.concatenate([bqkv[q_rows], bqkv[k_rows]]).reshape(E, 1)

        cons = np.zeros((1, CONS_LEN), np.float32)
        cons[0, ONES_OFF:ONES_OFF + 128] = 1.0
        cons[0, BV_OFF:BV_OFF + 512] = bqkv[v_rows]
        vpad = np.zeros(V1W, np.float32)
        for i in range(4):
            vpad[i * 65 + 64] = 1.0          # even-head ones col
            vpad[260 + i * 128] = 1.0        # odd-head ones col
        cons[0, VPAD_OFF:VPAD_OFF + V1W] = vpad

        wo = np.ascontiguousarray(Wout[:, dims].T)          # [512, E]
        bo = (bout if hh == 0 else np.zeros_like(bout)).reshape(E, 1)

        in_maps.append({
            "xw": xw, "bqk": np.ascontiguousarray(bqk),
            "cons": cons, "wo": wo, "bout": np.ascontiguousarray(bo),
        })
    return in_maps


def gather(results):
    out = np.empty((B, S, E), np.float32)
    for b in range(B):
        acc = results[2 * b]["outT"] + results[2 * b + 1]["outT"]   # [E, S]
        out[b] = acc.T
    return out


def kernel(query, key, value, Wqkv, bqkv, Wout, bout):
    # key/value are unused by the reference module (qkv all from query)
    nc = _get_nc()
    in_maps = make_in_maps(query, Wqkv, bqkv, Wout, bout)
    res = run_bass_kernel_spmd(nc, in_maps, list(range(8)))
    return gather(res.results)


# revision 7
# speedup vs baseline: 256.6675x; 1.0166x over previous
"""Multi-head attention (B=4, S=2048, E=1024, H=16) on 8 trn2 NeuronCores.

Sharding: data-parallel over B (4) x tensor-parallel over H (2 halves of 8
heads). Core c handles batch c//2, head-half c%2. Column-parallel qkv_proj,
row-parallel out_proj; the all-reduce of the two partial outputs per batch is
done on the host during unshard (a sum of two arrays), as is the final
transpose (the device emits out^T to keep DMA writes contiguous).

Device kernel (identical program on all 8 cores, fp32r matmuls):
  phase 1/2: qk^T = Wqk_loc @ x^T  [1024, 2048] and v = x @ Wv_loc^T + bv
             [2048, 512] (bias via a K=1 ones-row matmul)
  phase 3:   per head pair, flash-style over 128-key tiles:
             scores^T pairs row-packed at partitions 0/64, ACT exp with the
             1/sqrt(E) scale folded in, PV matmul with stationary [v | 1]
             (even head, M=65: ctx at partitions 0-63, sums at 64) or
             [1 | 0*63 | v] (odd head, M=128: sums at 0, ctx at 64-127) so
             the softmax denominator rides along for free; normalization by
             reciprocal of a DRAM-bounce partition-broadcast of the sums row
  phase 4:   out^T partial = Wout_loc^T-stationary matmuls + bias (bout on
             even cores only, zeros on odd, so the host sum adds it once)
"""
import sys

import numpy as np

sys.path.insert(0, "/opt/trn_rl_repo")

import concourse.bacc as bacc
import concourse.mybir as mybir
import concourse.tile as tile
from concourse.bass_utils import run_bass_kernel_spmd
from concourse.tile_rust import add_dep_helper

F32 = mybir.dt.float32
F32R = mybir.dt.float32r
EXP = mybir.ActivationFunctionType.Exp

B, S, E, H, HD = 4, 2048, 1024, 16, 64
HL = 8            # heads per core
SCALE = 1.0 / np.sqrt(E).astype(np.float32)

# cons layout: [0:128] ones, [128:640] bv, [640:1412] v1 pad pattern
ONES_OFF, BV_OFF, VPAD_OFF, CONS_LEN = 0, 128, 640, 1412
V1W = 4 * 65 + 4 * 128   # 772 cols per key-tile block in v1


def build_nc():
    nc = bacc.Bacc("TRN2", target_bir_lowering=False, debug=False, num_devices=8)
    xw_d = nc.declare_dram_parameter("xw", [E, 3584], F32, isOutput=False)
    bqk_d = nc.declare_dram_parameter("bqk", [E, 1], F32, isOutput=False)
    cons_d = nc.declare_dram_parameter("cons", [1, CONS_LEN], F32, isOutput=False)
    wo_d = nc.declare_dram_parameter("wo", [512, E], F32, isOutput=False)
    bout_d = nc.declare_dram_parameter("bout", [E, 1], F32, isOutput=False)
    out_d = nc.declare_dram_parameter("outT", [E, S], F32, isOutput=True)
    rb = nc.dram_tensor("rb", [8, S], F32)   # sums bounce rows

    with tile.TileContext(nc) as tc:
      with tc.tile_pool(name="pp", bufs=1) as pp:
        bqk_s = pp.tile([128, 8, 1], F32)
        bout_s = pp.tile([128, 8, 1], F32)
        cons_s = pp.tile([1, CONS_LEN], F32R)
        nc.gpsimd.dma_start(out=bqk_s, in_=bqk_d[:, :].rearrange("(m p) o -> p m o", p=128))
        nc.gpsimd.dma_start(out=bout_s, in_=bout_d[:, :].rearrange("(m p) o -> p m o", p=128))
        nc.gpsimd.dma_start(out=cons_s, in_=cons_d[:, :].bitcast(F32R))

        with tc.tile_pool(name="pa", bufs=1) as pa:
            qk_s = pa.tile([128, 8, S], F32R)     # qk^T: m-tile 0-3 q, 4-7 k
            v1_s = pa.tile([128, 16, V1W], F32R)  # per key-tile [v|1]x4, [1|0|v]x4

            # v1 pad pattern (ones + zero pads; v cols overwritten by evicts)
            for jt in range(16):
                nc.gpsimd.dma_start(
                    out=v1_s[:, jt, :],
                    in_=cons_d[0:1, VPAD_OFF:VPAD_OFF + V1W].bitcast(F32R)
                    .to_broadcast([128, V1W]))

            # ---- phases 1+2: qk^T and v
            with tc.tile_pool(name="p12", bufs=1) as p12, \
                 tc.tile_pool(name="ps12", bufs=1, space="PSUM") as ps12:
                wv_s = p12.tile([128, 8, 512], F32R)
                for mh in range(2):
                    wqk_s = p12.tile([128, 8, 512], F32R, tag="wqk", bufs=2)
                    for ic in range(4):
                        xc_s = p12.tile([128, 8, 512], F32R, tag="xc", bufs=2)
                        for kt in range(8):
                            if ic == 0:
                                nc.sync.dma_start(
                                    out=wqk_s[:, kt, :],
                                    in_=xw_d[kt * 128:(kt + 1) * 128,
                                             mh * 512:(mh + 1) * 512].bitcast(F32R))
                                if mh == 0:
                                    nc.gpsimd.dma_start(
                                        out=wv_s[:, kt, :],
                                        in_=xw_d[kt * 128:(kt + 1) * 128, 3072:3584]
                                        .bitcast(F32R))
                            nc.sync.dma_start(
                                out=xc_s[:, kt, :],
                                in_=xw_d[kt * 128:(kt + 1) * 128,
                                         1024 + ic * 512:1024 + (ic + 1) * 512]
                                .bitcast(F32R))
                        for m in range(4):
                            pq = ps12.tile([128, 512], F32, tag="pq", bufs=3)
                            for kt in range(8):
                                nc.tensor.matmul(
                                    out=pq, lhsT=wqk_s[:, kt, m * 128:(m + 1) * 128],
                                    rhs=xc_s[:, kt, :],
                                    start=(kt == 0), stop=(kt == 7))
                            nc.vector.tensor_scalar_add(
                                qk_s[:, mh * 4 + m, ic * 512:(ic + 1) * 512],
                                pq, bqk_s[:, mh * 4 + m, 0:1])
                        if mh == 0:
                            for st in range(4):
                                jt = ic * 4 + st
                                pv = ps12.tile([128, 512], F32, tag="pv", bufs=2)
                                for kt in range(8):
                                    nc.tensor.matmul(
                                        out=pv,
                                        lhsT=xc_s[:, kt, st * 128:(st + 1) * 128],
                                        rhs=wv_s[:, kt, :],
                                        start=(kt == 0), stop=False)
                                nc.tensor.matmul(
                                    out=pv, lhsT=cons_s[0:1, ONES_OFF:ONES_OFF + 128],
                                    rhs=cons_s[0:1, BV_OFF:BV_OFF + 512],
                                    start=False, stop=True)
                                # evict: even heads -> [v|1] blocks, odd -> [1|0|v]
                                nc.vector.tensor_copy(
                                    v1_s[:, jt, 0:260]
                                    .rearrange("p (b c) -> p b c", c=65)[:, :, 0:64],
                                    pv[:, :].rearrange("p (b t d) -> p b t d", t=2, d=64)
                                    [:, :, 0, :])
                                nc.vector.tensor_copy(
                                    v1_s[:, jt, 260:V1W]
                                    .rearrange("p (b c) -> p b c", c=128)[:, :, 64:128],
                                    pv[:, :].rearrange("p (b t d) -> p b t d", t=2, d=64)
                                    [:, :, 1, :])

            # ---- phase 3: attention per head pair
            with tc.tile_pool(name="pc", bufs=1) as pc:
                ctx_t = [pc.tile([128, S], F32R, name=f"ctx{i}", tag=f"ctx{i}",
                                 bufs=1) for i in range(4)]
                wo_s = pc.tile([128, 4, E], F32R)
                for ct in range(4):
                    nc.sync.dma_start(
                        out=wo_s[:, ct, :],
                        in_=wo_d[ct * 128:(ct + 1) * 128, :].bitcast(F32R))
                with tc.tile_pool(name="ps3", bufs=1, space="PSUM") as ps3:
                  with tc.tile_pool(name="p3", bufs=1) as p3:
                    for p in range(4):
                        for icp in range(2):
                            s_e = ps3.tile([128, 1024], F32, tag="s_e", bufs=1)
                            s_o = ps3.tile([128, 1024], F32, tag="s_o", bufs=1)
                            pv_e = ps3.tile([65, 1024], F32, tag="pv_e", bufs=1)
                            pv_o = ps3.tile([128, 1024], F32, tag="pv_o", bufs=1)
                            for jt in range(16):
                                for ih in range(2):
                                    icol = icp * 1024 + ih * 512
                                    nc.tensor.matmul(
                                        out=s_e[:, ih * 512:(ih + 1) * 512],
                                        lhsT=qk_s[0:64, 4 + p, jt * 128:(jt + 1) * 128],
                                        rhs=qk_s[0:64, p, icol:icol + 512],
                                        start=True, stop=True)
                                    nc.tensor.matmul(
                                        out=s_o[:, ih * 512:(ih + 1) * 512],
                                        lhsT=qk_s[64:128, 4 + p, jt * 128:(jt + 1) * 128],
                                        rhs=qk_s[64:128, p, icol:icol + 512],
                                        start=True, stop=True)
                                e_e = p3.tile([128, 1024], F32R, tag="e", bufs=2)
                                nc.scalar.activation(out=e_e, in_=s_e, func=EXP,
                                                     scale=float(SCALE))
                                e_o = p3.tile([128, 1024], F32R, tag="e", bufs=2)
                                nc.scalar.activation(out=e_o, in_=s_o, func=EXP,
                                                     scale=float(SCALE))
                                for ih in range(2):
                                    sl = slice(ih * 512, (ih + 1) * 512)
                                    nc.tensor.matmul(
                                        out=pv_e[:, sl],
                                        lhsT=v1_s[:, jt, p * 65:p * 65 + 65],
                                        rhs=e_e[:, sl],
                                        start=(jt == 0), stop=(jt == 15))
                                    nc.tensor.matmul(
                                        out=pv_o[:, sl],
                                        lhsT=v1_s[:, jt, 260 + p * 128:260 + (p + 1) * 128],
                                        rhs=e_o[:, sl],
                                        start=(jt == 0), stop=(jt == 15))
                            # evict pv to sbuf (frees psum), then normalize
                            pvt_e = p3.tile([65, 1024], F32, tag="pvt_e", bufs=1)
                            pvt_o = p3.tile([128, 1024], F32, tag="pvt_o", bufs=1)
                            nc.vector.tensor_copy(pvt_e, pv_e)
                            nc.vector.tensor_copy(pvt_o, pv_o)
                            ic_sl = slice(icp * 1024, (icp + 1) * 1024)
                            st_e = nc.sync.dma_start(out=rb[2 * p:2 * p + 1, ic_sl],
                                                     in_=pvt_e[64:65, :])
                            st_o = nc.sync.dma_start(out=rb[2 * p + 1:2 * p + 2, ic_sl],
                                                     in_=pvt_o[0:1, :])
                            rep = p3.tile([128, 1024], F32, tag="rep", bufs=1)
                            ld_e = nc.gpsimd.dma_start(
                                out=rep[0:64, :],
                                in_=rb[2 * p:2 * p + 1, ic_sl].to_broadcast([64, 1024]))
                            ld_o = nc.gpsimd.dma_start(
                                out=rep[64:128, :],
                                in_=rb[2 * p + 1:2 * p + 2, ic_sl].to_broadcast([64, 1024]))
                            add_dep_helper(ld_e.ins, st_e.ins, sync=True, reason="raw_e")
                            add_dep_helper(ld_o.ins, st_o.ins, sync=True, reason="raw_o")
                            rrec = p3.tile([128, 1024], F32, tag="rrec", bufs=1)
                            rscr = p3.tile([128, 1024], F32, tag="rscr", bufs=1)
                            nc.vector.reciprocal_approx_accurate(
                                out=rrec, in_=rep, scratch=rscr)
                            nc.vector.tensor_mul(ctx_t[p][0:64, ic_sl],
                                                 pvt_e[0:64, :], rrec[0:64, :])
                            nc.vector.tensor_mul(ctx_t[p][64:128, ic_sl],
                                                 pvt_o[64:128, :], rrec[64:128, :])

                  # ---- phase 4: out projection (partial), written as out^T
                  # (still inside ps3: po reuses the s_e/s_o tag slots so there
                  # is no psum pool transition barrier)
                  with tc.tile_pool(name="p4", bufs=1) as p4:
                    for et in range(8):
                        for i4 in range(4):
                            k = et * 4 + i4
                            po = ps3.tile([128, 512], F32,
                                          tag=("s_e" if k % 2 == 0 else "s_o"),
                                          bufs=1, name=f"po_{k}")
                            for ct in range(4):
                                nc.tensor.matmul(
                                    out=po, lhsT=wo_s[:, ct, et * 128:(et + 1) * 128],
                                    rhs=ctx_t[ct][:, i4 * 512:(i4 + 1) * 512],
                                    start=(ct == 0), stop=(ct == 3))
                            ot = p4.tile([128, 512], F32, tag="ot", bufs=4)
                            nc.vector.tensor_scalar_add(ot, po, bout_s[:, et, 0:1])
                            nc.sync.dma_start(
                                out=out_d[et * 128:(et + 1) * 128,
                                          i4 * 512:(i4 + 1) * 512],
                                in_=ot)
    nc.compile()
    return nc


_NC = None


def _get_nc():
    global _NC
    if _NC is None:
        _NC = build_nc()
    return _NC


def make_in_maps(query, Wqkv, bqkv, Wout, bout):
    query = np.asarray(query, dtype=np.float32)
    Wqkv = np.asarray(Wqkv, dtype=np.float32)
    bqkv = np.asarray(bqkv, dtype=np.float32)
    Wout = np.asarray(Wout, dtype=np.float32)
    bout = np.asarray(bout, dtype=np.float32)

    in_maps = []
    for c in range(8):
        b, hh = c // 2, c % 2
        heads = np.arange(hh * HL, hh * HL + HL)
        dims = (heads[:, None] * HD + np.arange(HD)[None, :]).reshape(-1)  # [512]
        q_rows, k_rows, v_rows = dims, E + dims, 2 * E + dims

        xw = np.empty((E, 3584), np.float32)
        xw[:, 0:512] = Wqkv[q_rows].T
        xw[:, 512:1024] = Wqkv[k_rows].T
        xw[:, 1024:3072] = query[b].T
        xw[:, 3072:3584] = Wqkv[v_rows].T

        bqk = np.concatenate([bqkv[q_rows], bqkv[k_rows]]).reshape(E, 1)

        cons = np.zeros((1, CONS_LEN), np.float32)
        cons[0, ONES_OFF:ONES_OFF + 128] = 1.0
        cons[0, BV_OFF:BV_OFF + 512] = bqkv[v_rows]
        vpad = np.zeros(V1W, np.float32)
        for i in range(4):
            vpad[i * 65 + 64] = 1.0          # even-head ones col
            vpad[260 + i * 128] = 1.0        # odd-head ones col
        cons[0, VPAD_OFF:VPAD_OFF + V1W] = vpad

        wo = np.ascontiguousarray(Wout[:, dims].T)          # [512, E]
        bo = (bout if hh == 0 else np.zeros_like(bout)).reshape(E, 1)

        in_maps.append({
            "xw": xw, "bqk": np.ascontiguousarray(bqk),
            "cons": cons, "wo": wo, "bout": np.ascontiguousarray(bo),
        })
    return in_maps


def gather(results):
    out = np.empty((B, S, E), np.float32)
    for b in range(B):
        acc = results[2 * b]["outT"] + results[2 * b + 1]["outT"]   # [E, S]
        out[b] = acc.T
    return out


def kernel(query, key, value, Wqkv, bqkv, Wout, bout):
    # key/value are unused by the reference module (qkv all from query)
    nc = _get_nc()
    in_maps = make_in_maps(query, Wqkv, bqkv, Wout, bout)
    res = run_bass_kernel_spmd(nc, in_maps, list(range(8)))
    return gather(res.results)


# revision 11
# speedup vs baseline: 258.5304x; 1.0073x over previous
"""Multi-head attention (B=4, S=2048, E=1024, H=16) on 8 trn2 NeuronCores.

Sharding: data-parallel over B (4) x tensor-parallel over H (2 halves of 8
heads). Core c handles batch c//2, head-half c%2. Column-parallel qkv_proj,
row-parallel out_proj; the all-reduce of the two partial outputs per batch is
done on the host during unshard (a sum of two arrays), as is the final
transpose (the device emits out^T to keep DMA writes contiguous).

Device kernel (identical program on all 8 cores, fp32r matmuls):
  phase 1/2: qk^T = Wqk_loc @ x^T  [1024, 2048] and v = x @ Wv_loc^T + bv
             [2048, 512] (bias via a K=1 ones-row matmul)
  phase 3:   per head pair, flash-style over 128-key tiles:
             scores^T pairs row-packed at partitions 0/64, ACT exp with the
             1/sqrt(E) scale folded in, PV matmul with stationary [v | 1]
             (even head, M=65: ctx at partitions 0-63, sums at 64) or
             [1 | 0*63 | v] (odd head, M=128: sums at 0, ctx at 64-127) so
             the softmax denominator rides along for free; normalization by
             reciprocal of a DRAM-bounce partition-broadcast of the sums row
  phase 4:   out^T partial = Wout_loc^T-stationary matmuls + bias (bout on
             even cores only, zeros on odd, so the host sum adds it once)
"""
import sys

import numpy as np

sys.path.insert(0, "/opt/trn_rl_repo")

import concourse.bacc as bacc
import concourse.mybir as mybir
import concourse.tile as tile
from concourse.bass_utils import run_bass_kernel_spmd
from concourse.tile_rust import add_dep_helper

F32 = mybir.dt.float32
F32R = mybir.dt.float32r
EXP = mybir.ActivationFunctionType.Exp

B, S, E, H, HD = 4, 2048, 1024, 16, 64
HL = 8            # heads per core
SCALE = 1.0 / np.sqrt(E).astype(np.float32)

# cons layout: [0:128] ones, [128:640] bv, [640:1412] v1 pad pattern
ONES_OFF, BV_OFF, VPAD_OFF, CONS_LEN = 0, 128, 640, 1412
V1W = 4 * 65 + 4 * 128   # 772 cols per key-tile block in v1


def build_nc():
    nc = bacc.Bacc("TRN2", target_bir_lowering=False, debug=False, num_devices=8)
    xw_d = nc.declare_dram_parameter("xw", [E, 3584], F32, isOutput=False)
    bqk_d = nc.declare_dram_parameter("bqk", [E, 1], F32, isOutput=False)
    cons_d = nc.declare_dram_parameter("cons", [1, CONS_LEN], F32, isOutput=False)
    wo_d = nc.declare_dram_parameter("wo", [512, E], F32, isOutput=False)
    bout_d = nc.declare_dram_parameter("bout", [E, 1], F32, isOutput=False)
    out_d = nc.declare_dram_parameter("outT", [E, S], F32, isOutput=True)
    rb = nc.dram_tensor("rb", [8, S], F32)   # sums bounce rows

    with tile.TileContext(nc) as tc:
      with tc.tile_pool(name="pp", bufs=1) as pp:
        bqk_s = pp.tile([128, 8, 1], F32)
        bout_s = pp.tile([128, 8, 1], F32)
        cons_s = pp.tile([1, CONS_LEN], F32R)
        nc.gpsimd.dma_start(out=bqk_s, in_=bqk_d[:, :].rearrange("(m p) o -> p m o", p=128))
        nc.gpsimd.dma_start(out=bout_s, in_=bout_d[:, :].rearrange("(m p) o -> p m o", p=128))
        nc.gpsimd.dma_start(out=cons_s, in_=cons_d[:, :].bitcast(F32R))

        with tc.tile_pool(name="pa", bufs=1) as pa:
            qk_s = pa.tile([128, 8, S], F32R)     # qk^T: m-tile 0-3 q, 4-7 k
            v1_s = pa.tile([128, 16, V1W], F32R)  # per key-tile [v|1]x4, [1|0|v]x4

            # v1 pad pattern (ones + zero pads; v cols overwritten by evicts)
            for jt in range(16):
                nc.gpsimd.dma_start(
                    out=v1_s[:, jt, :],
                    in_=cons_d[0:1, VPAD_OFF:VPAD_OFF + V1W].bitcast(F32R)
                    .to_broadcast([128, V1W]))

            # ---- phases 1+2: qk^T and v
            with tc.tile_pool(name="p12", bufs=1) as p12, \
                 tc.tile_pool(name="ps12", bufs=1, space="PSUM") as ps12:
                wv_s = p12.tile([128, 8, 512], F32R)
                for mh in range(2):
                    wqk_s = p12.tile([128, 8, 512], F32R, tag="wqk", bufs=2)
                    for ic in range(4):
                        xc_s = p12.tile([128, 8, 512], F32R, tag="xc", bufs=2)
                        for kt in range(8):
                            if ic == 0:
                                nc.sync.dma_start(
                                    out=wqk_s[:, kt, :],
                                    in_=xw_d[kt * 128:(kt + 1) * 128,
                                             mh * 512:(mh + 1) * 512].bitcast(F32R))
                                if mh == 0:
                                    nc.gpsimd.dma_start(
                                        out=wv_s[:, kt, :],
                                        in_=xw_d[kt * 128:(kt + 1) * 128, 3072:3584]
                                        .bitcast(F32R))
                            nc.sync.dma_start(
                                out=xc_s[:, kt, :],
                                in_=xw_d[kt * 128:(kt + 1) * 128,
                                         1024 + ic * 512:1024 + (ic + 1) * 512]
                                .bitcast(F32R))
                        for m in range(4):
                            pq = ps12.tile([128, 512], F32, tag="pq", bufs=4)
                            for kt in range(8):
                                nc.tensor.matmul(
                                    out=pq, lhsT=wqk_s[:, kt, m * 128:(m + 1) * 128],
                                    rhs=xc_s[:, kt, :],
                                    start=(kt == 0), stop=(kt == 7))
                            nc.vector.tensor_scalar_add(
                                qk_s[:, mh * 4 + m, ic * 512:(ic + 1) * 512],
                                pq, bqk_s[:, mh * 4 + m, 0:1])
                        if mh == 0:
                            for st in range(4):
                                jt = ic * 4 + st
                                pv = ps12.tile([128, 512], F32, tag="pv", bufs=3)
                                for kt in range(8):
                                    nc.tensor.matmul(
                                        out=pv,
                                        lhsT=xc_s[:, kt, st * 128:(st + 1) * 128],
                                        rhs=wv_s[:, kt, :],
                                        start=(kt == 0), stop=False)
                                nc.tensor.matmul(
                                    out=pv, lhsT=cons_s[0:1, ONES_OFF:ONES_OFF + 128],
                                    rhs=cons_s[0:1, BV_OFF:BV_OFF + 512],
                                    start=False, stop=True)
                                # evict: even heads -> [v|1] blocks, odd -> [1|0|v]
                                nc.vector.tensor_copy(
                                    v1_s[:, jt, 0:260]
                                    .rearrange("p (b c) -> p b c", c=65)[:, :, 0:64],
                                    pv[:, :].rearrange("p (b t d) -> p b t d", t=2, d=64)
                                    [:, :, 0, :])
                                nc.vector.tensor_copy(
                                    v1_s[:, jt, 260:V1W]
                                    .rearrange("p (b c) -> p b c", c=128)[:, :, 64:128],
                                    pv[:, :].rearrange("p (b t d) -> p b t d", t=2, d=64)
                                    [:, :, 1, :])

            # ---- phase 3: attention per head pair
            with tc.tile_pool(name="pc", bufs=1) as pc:
                ctx_t = [pc.tile([128, S], F32R, name=f"ctx{i}", tag=f"ctx{i}",
                                 bufs=1) for i in range(4)]
                wo_s = pc.tile([128, 4, E], F32R)
                for ct in range(4):
                    nc.sync.dma_start(
                        out=wo_s[:, ct, :],
                        in_=wo_d[ct * 128:(ct + 1) * 128, :].bitcast(F32R))
                with tc.tile_pool(name="ps3", bufs=1, space="PSUM") as ps3:
                  with tc.tile_pool(name="p3", bufs=1) as p3:
                    for p in range(4):
                        for icp in range(2):
                            s_e = ps3.tile([128, 1024], F32, tag="s_e", bufs=1)
                            s_o = ps3.tile([128, 1024], F32, tag="s_o", bufs=1)
                            pv_e = ps3.tile([65, 1024], F32, tag="pv_e", bufs=1)
                            pv_o = ps3.tile([128, 1024], F32, tag="pv_o", bufs=1)
                            for jt in range(16):
                                for ih in range(2):
                                    icol = icp * 1024 + ih * 512
                                    nc.tensor.matmul(
                                        out=s_e[:, ih * 512:(ih + 1) * 512],
                                        lhsT=qk_s[0:64, 4 + p, jt * 128:(jt + 1) * 128],
                                        rhs=qk_s[0:64, p, icol:icol + 512],
                                        start=True, stop=True)
                                    nc.tensor.matmul(
                                        out=s_o[:, ih * 512:(ih + 1) * 512],
                                        lhsT=qk_s[64:128, 4 + p, jt * 128:(jt + 1) * 128],
                                        rhs=qk_s[64:128, p, icol:icol + 512],
                                        start=True, stop=True)
                                e_e = p3.tile([128, 1024], F32R, tag="e", bufs=2)
                                nc.scalar.activation(out=e_e, in_=s_e, func=EXP,
                                                     scale=float(SCALE))
                                e_o = p3.tile([128, 1024], F32R, tag="e", bufs=2)
                                nc.scalar.activation(out=e_o, in_=s_o, func=EXP,
                                                     scale=float(SCALE))
                                for ih in range(2):
                                    sl = slice(ih * 512, (ih + 1) * 512)
                                    nc.tensor.matmul(
                                        out=pv_e[:, sl],
                                        lhsT=v1_s[:, jt, p * 65:p * 65 + 65],
                                        rhs=e_e[:, sl],
                                        start=(jt == 0), stop=(jt == 15))
                                    nc.tensor.matmul(
                                        out=pv_o[:, sl],
                                        lhsT=v1_s[:, jt, 260 + p * 128:260 + (p + 1) * 128],
                                        rhs=e_o[:, sl],
                                        start=(jt == 0), stop=(jt == 15))
                            # evict pv to sbuf (frees psum), then normalize
                            pvt_e = p3.tile([65, 1024], F32, tag="pvt_e", bufs=1)
                            pvt_o = p3.tile([128, 1024], F32, tag="pvt_o", bufs=1)
                            nc.vector.tensor_copy(pvt_e, pv_e)
                            nc.vector.tensor_copy(pvt_o, pv_o)
                            ic_sl = slice(icp * 1024, (icp + 1) * 1024)
                            st_e = nc.sync.dma_start(out=rb[2 * p:2 * p + 1, ic_sl],
                                                     in_=pvt_e[64:65, :])
                            st_o = nc.sync.dma_start(out=rb[2 * p + 1:2 * p + 2, ic_sl],
                                                     in_=pvt_o[0:1, :])
                            rep = p3.tile([128, 1024], F32, tag="rep", bufs=1)
                            ld_e = nc.gpsimd.dma_start(
                                out=rep[0:64, :],
                                in_=rb[2 * p:2 * p + 1, ic_sl].to_broadcast([64, 1024]))
                            ld_o = nc.gpsimd.dma_start(
                                out=rep[64:128, :],
                                in_=rb[2 * p + 1:2 * p + 2, ic_sl].to_broadcast([64, 1024]))
                            add_dep_helper(ld_e.ins, st_e.ins, sync=True, reason="raw_e")
                            add_dep_helper(ld_o.ins, st_o.ins, sync=True, reason="raw_o")
                            rrec = p3.tile([128, 1024], F32, tag="rrec", bufs=1)
                            rscr = p3.tile([128, 1024], F32, tag="rscr", bufs=1)
                            nc.vector.reciprocal_approx_accurate(
                                out=rrec, in_=rep, scratch=rscr)
                            nc.vector.tensor_mul(ctx_t[p][0:64, ic_sl],
                                                 pvt_e[0:64, :], rrec[0:64, :])
                            nc.vector.tensor_mul(ctx_t[p][64:128, ic_sl],
                                                 pvt_o[64:128, :], rrec[64:128, :])

                  # ---- phase 4: out projection (partial), written as out^T
                  # (still inside ps3: po reuses the s_e/s_o tag slots so there
                  # is no psum pool transition barrier)
                  with tc.tile_pool(name="p4", bufs=1) as p4:
                    for et in range(8):
                        for i4 in range(4):
                            k = et * 4 + i4
                            po = ps3.tile([128, 512], F32,
                                          tag=("s_e" if k % 2 == 0 else "s_o"),
                                          bufs=1, name=f"po_{k}")
                            for ct in range(4):
                                nc.tensor.matmul(
                                    out=po, lhsT=wo_s[:, ct, et * 128:(et + 1) * 128],
                                    rhs=ctx_t[ct][:, i4 * 512:(i4 + 1) * 512],
                                    start=(ct == 0), stop=(ct == 3))
                            ot = p4.tile([128, 512], F32, tag="ot", bufs=4)
                            nc.vector.tensor_scalar_add(ot, po, bout_s[:, et, 0:1])
                            nc.sync.dma_start(
                                out=out_d[et * 128:(et + 1) * 128,
                                          i4 * 512:(i4 + 1) * 512],
                                in_=ot)
    nc.compile()
    return nc


_NC = None


def _get_nc():
    global _NC
    if _NC is None:
        _NC = build_nc()
    return _NC


def make_in_maps(query, Wqkv, bqkv, Wout, bout):
    query = np.asarray(query, dtype=np.float32)
    Wqkv = np.asarray(Wqkv, dtype=np.float32)
    bqkv = np.asarray(bqkv, dtype=np.float32)
    Wout = np.asarray(Wout, dtype=np.float32)
    bout = np.asarray(bout, dtype=np.float32)

    in_maps = []
    for c in range(8):
        b, hh = c // 2, c % 2
        heads = np.arange(hh * HL, hh * HL + HL)
        dims = (heads[:, None] * HD + np.arange(HD)[None, :]).reshape(-1)  # [512]
        q_rows, k_rows, v_rows = dims, E + dims, 2 * E + dims

        xw = np.empty((E, 3584), np.float32)
        xw[:, 0:512] = Wqkv[q_rows].T
        xw[:, 512:1024] = Wqkv[k_rows].T
        xw[:, 1024:3072] = query[b].T
        xw[:, 3072:3584] = Wqkv[v_rows].T

        bqk = np.concatenate([bqkv[q_rows], bqkv[k_rows]]).reshape(E, 1)

        cons = np.zeros((1, CONS_LEN), np.float32)
        cons[0, ONES_OFF:ONES_OFF + 128] = 1.0
        cons[0, BV_OFF:BV_OFF + 512] = bqkv[v_rows]
        vpad = np.zeros(V1W, np.float32)
        for i in range(4):
            vpad[i * 65 + 64] = 1.0          # even-head ones col
            vpad[260 + i * 128] = 1.0        # odd-head ones col
        cons[0, VPAD_OFF:VPAD_OFF + V1W] = vpad

        wo = np.ascontiguousarray(Wout[:, dims].T)          # [512, E]
        bo = (bout if hh == 0 else np.zeros_like(bout)).reshape(E, 1)

        in_maps.append({
            "xw": xw, "bqk": np.ascontiguousarray(bqk),
            "cons": cons, "wo": wo, "bout": np.ascontiguousarray(bo),
        })
    return in_maps


def gather(results):
    out = np.empty((B, S, E), np.float32)
    for b in range(B):
        acc = results[2 * b]["outT"] + results[2 * b + 1]["outT"]   # [E, S]
        out[b] = acc.T
    return out


def kernel(query, key, value, Wqkv, bqkv, Wout, bout):
    # key/value are unused by the reference module (qkv all from query)
    nc = _get_nc()
    in_maps = make_in_maps(query, Wqkv, bqkv, Wout, bout)
    res = run_bass_kernel_spmd(nc, in_maps, list(range(8)))
    return gather(res.results)


# revision 14
# speedup vs baseline: 262.7256x; 1.0162x over previous
"""Multi-head attention (B=4, S=2048, E=1024, H=16) on 8 trn2 NeuronCores.

Sharding: data-parallel over B (4) x tensor-parallel over H (2 halves of 8
heads). Core c handles batch c//2, head-half c%2. Column-parallel qkv_proj,
row-parallel out_proj; the all-reduce of the two partial outputs per batch is
done on the host during unshard (a sum of two arrays), as is the final
transpose (the device emits out^T to keep DMA writes contiguous).

Device kernel (identical program on all 8 cores, fp32r matmuls):
  phase 1/2: qk^T = Wqk_loc @ x^T  [1024, 2048] and v = x @ Wv_loc^T + bv
             [2048, 512] (bias via a K=1 ones-row matmul)
  phase 3:   per head pair, flash-style over 128-key tiles:
             scores^T pairs row-packed at partitions 0/64, ACT exp with the
             1/sqrt(E) scale folded in, PV matmul with stationary [v | 1]
             (even head, M=65: ctx at partitions 0-63, sums at 64) or
             [1 | 0*63 | v] (odd head, M=128: sums at 0, ctx at 64-127) so
             the softmax denominator rides along for free; normalization by
             reciprocal of a DRAM-bounce partition-broadcast of the sums row
  phase 4:   out^T partial = Wout_loc^T-stationary matmuls + bias (bout on
             even cores only, zeros on odd, so the host sum adds it once)
"""
import sys

import numpy as np

sys.path.insert(0, "/opt/trn_rl_repo")

import concourse.bacc as bacc
import concourse.mybir as mybir
import concourse.tile as tile
from concourse.bass_utils import run_bass_kernel_spmd
from concourse.tile_rust import add_dep_helper

F32 = mybir.dt.float32
F32R = mybir.dt.float32r
EXP = mybir.ActivationFunctionType.Exp

B, S, E, H, HD = 4, 2048, 1024, 16, 64
HL = 8            # heads per core
SCALE = 1.0 / np.sqrt(E).astype(np.float32)

# cons layout: [0:128] ones, [128:640] bv, [640:1412] v1 pad pattern
ONES_OFF, BV_OFF, VPAD_OFF, CONS_LEN = 0, 128, 640, 1412
V1W = 4 * 65 + 4 * 128   # 772 cols per key-tile block in v1


def build_nc():
    nc = bacc.Bacc("TRN2", target_bir_lowering=False, debug=False, num_devices=8)
    xw_d = nc.declare_dram_parameter("xw", [E, 3584], F32, isOutput=False)
    bqk_d = nc.declare_dram_parameter("bqk", [E, 1], F32, isOutput=False)
    cons_d = nc.declare_dram_parameter("cons", [1, CONS_LEN], F32, isOutput=False)
    wo_d = nc.declare_dram_parameter("wo", [512, E], F32, isOutput=False)
    bout_d = nc.declare_dram_parameter("bout", [E, 1], F32, isOutput=False)
    out_d = nc.declare_dram_parameter("outT", [E, S], F32, isOutput=True)
    rb = nc.dram_tensor("rb", [8, S], F32)   # sums bounce rows

    with tile.TileContext(nc) as tc:
      with tc.tile_pool(name="pp", bufs=1) as pp:
        bqk_s = pp.tile([128, 8, 1], F32)
        bout_s = pp.tile([128, 8, 1], F32)
        cons_s = pp.tile([1, CONS_LEN], F32R)
        nc.gpsimd.dma_start(out=bqk_s, in_=bqk_d[:, :].rearrange("(m p) o -> p m o", p=128))
        nc.gpsimd.dma_start(out=bout_s, in_=bout_d[:, :].rearrange("(m p) o -> p m o", p=128))
        nc.gpsimd.dma_start(out=cons_s, in_=cons_d[:, :].bitcast(F32R))

        with tc.tile_pool(name="pa", bufs=1) as pa:
            qk_s = pa.tile([128, 8, S], F32R)     # qk^T: m-tile 0-3 q, 4-7 k
            v1_s = pa.tile([128, 16, V1W], F32R)  # per key-tile [v|1]x4, [1|0|v]x4

            # ---- phases 1+2: qk^T and v
            with tc.tile_pool(name="p12", bufs=1) as p12, \
                 tc.tile_pool(name="ps12", bufs=1, space="PSUM") as ps12:
                wv_s = p12.tile([128, 8, 512], F32R)
                for kt in range(8):
                    nc.gpsimd.dma_start(
                        out=wv_s[:, kt, :],
                        in_=xw_d[kt * 128:(kt + 1) * 128, 3072:3584].bitcast(F32R))
                # v1 pad pattern (ones + zero pads; v cols overwritten by evicts)
                # -- after wv on the gpsimd queue: wv is needed ~15us earlier
                for jt in range(16):
                    nc.gpsimd.dma_start(
                        out=v1_s[:, jt, :],
                        in_=cons_d[0:1, VPAD_OFF:VPAD_OFF + V1W].bitcast(F32R)
                        .to_broadcast([128, V1W]))
                for mh in range(2):
                    wqk_s = p12.tile([128, 8, 512], F32R, tag="wqk", bufs=2)
                    for ic in range(4):
                        xc_s = p12.tile([128, 8, 512], F32R, tag="xc", bufs=2)
                        for kt in range(8):
                            if ic == 0:
                                nc.sync.dma_start(
                                    out=wqk_s[:, kt, :],
                                    in_=xw_d[kt * 128:(kt + 1) * 128,
                                             mh * 512:(mh + 1) * 512].bitcast(F32R))
                            nc.sync.dma_start(
                                out=xc_s[:, kt, :],
                                in_=xw_d[kt * 128:(kt + 1) * 128,
                                         1024 + ic * 512:1024 + (ic + 1) * 512]
                                .bitcast(F32R))
                        for m in range(4):
                            pq = ps12.tile([128, 512], F32, tag="pq", bufs=4)
                            for kt in range(8):
                                nc.tensor.matmul(
                                    out=pq, lhsT=wqk_s[:, kt, m * 128:(m + 1) * 128],
                                    rhs=xc_s[:, kt, :],
                                    start=(kt == 0), stop=(kt == 7))
                            nc.vector.tensor_scalar_add(
                                qk_s[:, mh * 4 + m, ic * 512:(ic + 1) * 512],
                                pq, bqk_s[:, mh * 4 + m, 0:1])
                        if mh == 0:
                            for st in range(4):
                                jt = ic * 4 + st
                                pv = ps12.tile([128, 512], F32, tag="pv", bufs=3)
                                for kt in range(8):
                                    nc.tensor.matmul(
                                        out=pv,
                                        lhsT=xc_s[:, kt, st * 128:(st + 1) * 128],
                                        rhs=wv_s[:, kt, :],
                                        start=(kt == 0), stop=False)
                                nc.tensor.matmul(
                                    out=pv, lhsT=cons_s[0:1, ONES_OFF:ONES_OFF + 128],
                                    rhs=cons_s[0:1, BV_OFF:BV_OFF + 512],
                                    start=False, stop=True)
                                # evict: even heads -> [v|1] blocks, odd -> [1|0|v]
                                nc.vector.tensor_copy(
                                    v1_s[:, jt, 0:260]
                                    .rearrange("p (b c) -> p b c", c=65)[:, :, 0:64],
                                    pv[:, :].rearrange("p (b t d) -> p b t d", t=2, d=64)
                                    [:, :, 0, :])
                                nc.vector.tensor_copy(
                                    v1_s[:, jt, 260:V1W]
                                    .rearrange("p (b c) -> p b c", c=128)[:, :, 64:128],
                                    pv[:, :].rearrange("p (b t d) -> p b t d", t=2, d=64)
                                    [:, :, 1, :])

            # ---- phase 3: attention per head pair
            with tc.tile_pool(name="pc", bufs=1) as pc:
                ctx_t = [pc.tile([128, S], F32R, name=f"ctx{i}", tag=f"ctx{i}",
                                 bufs=1) for i in range(4)]
                wo_s = pc.tile([128, 4, E], F32R)
                for ct in range(4):
                    nc.sync.dma_start(
                        out=wo_s[:, ct, :],
                        in_=wo_d[ct * 128:(ct + 1) * 128, :].bitcast(F32R))
                with tc.tile_pool(name="ps3", bufs=1, space="PSUM") as ps3:
                  with tc.tile_pool(name="p3", bufs=1) as p3:
                    for p in range(4):
                        for icp in range(2):
                            s_e = ps3.tile([128, 1024], F32, tag="s_e", bufs=1)
                            s_o = ps3.tile([128, 1024], F32, tag="s_o", bufs=1)
                            pv_e = ps3.tile([65, 1024], F32, tag="pv_e", bufs=1)
                            pv_o = ps3.tile([128, 1024], F32, tag="pv_o", bufs=1)
                            for jt in range(16):
                                for ih in range(2):
                                    icol = icp * 1024 + ih * 512
                                    nc.tensor.matmul(
                                        out=s_e[:, ih * 512:(ih + 1) * 512],
                                        lhsT=qk_s[0:64, 4 + p, jt * 128:(jt + 1) * 128],
                                        rhs=qk_s[0:64, p, icol:icol + 512],
                                        start=True, stop=True)
                                    nc.tensor.matmul(
                                        out=s_o[:, ih * 512:(ih + 1) * 512],
                                        lhsT=qk_s[64:128, 4 + p, jt * 128:(jt + 1) * 128],
                                        rhs=qk_s[64:128, p, icol:icol + 512],
                                        start=True, stop=True)
                                e_e = p3.tile([128, 1024], F32R, tag="e", bufs=2)
                                nc.scalar.activation(out=e_e, in_=s_e, func=EXP,
                                                     scale=float(SCALE))
                                e_o = p3.tile([128, 1024], F32R, tag="e", bufs=2)
                                nc.scalar.activation(out=e_o, in_=s_o, func=EXP,
                                                     scale=float(SCALE))
                                for ih in range(2):
                                    sl = slice(ih * 512, (ih + 1) * 512)
                                    nc.tensor.matmul(
                                        out=pv_e[:, sl],
                                        lhsT=v1_s[:, jt, p * 65:p * 65 + 65],
                                        rhs=e_e[:, sl],
                                        start=(jt == 0), stop=(jt == 15))
                                    nc.tensor.matmul(
                                        out=pv_o[:, sl],
                                        lhsT=v1_s[:, jt, 260 + p * 128:260 + (p + 1) * 128],
                                        rhs=e_o[:, sl],
                                        start=(jt == 0), stop=(jt == 15))
                            # evict pv to sbuf (frees psum), then normalize
                            pvt_e = p3.tile([65, 1024], F32, tag="pvt_e", bufs=1)
                            pvt_o = p3.tile([128, 1024], F32, tag="pvt_o", bufs=1)
                            nc.vector.tensor_copy(pvt_e, pv_e)
                            nc.vector.tensor_copy(pvt_o, pv_o)
                            ic_sl = slice(icp * 1024, (icp + 1) * 1024)
                            st_e = nc.sync.dma_start(out=rb[2 * p:2 * p + 1, ic_sl],
                                                     in_=pvt_e[64:65, :])
                            st_o = nc.sync.dma_start(out=rb[2 * p + 1:2 * p + 2, ic_sl],
                                                     in_=pvt_o[0:1, :])
                            rep = p3.tile([128, 1024], F32, tag="rep", bufs=1)
                            ld_e = nc.gpsimd.dma_start(
                                out=rep[0:64, :],
                                in_=rb[2 * p:2 * p + 1, ic_sl].to_broadcast([64, 1024]))
                            ld_o = nc.gpsimd.dma_start(
                                out=rep[64:128, :],
                                in_=rb[2 * p + 1:2 * p + 2, ic_sl].to_broadcast([64, 1024]))
                            add_dep_helper(ld_e.ins, st_e.ins, sync=True, reason="raw_e")
                            add_dep_helper(ld_o.ins, st_o.ins, sync=True, reason="raw_o")
                            rrec = p3.tile([128, 1024], F32, tag="rrec", bufs=1)
                            rscr = p3.tile([128, 1024], F32, tag="rscr", bufs=1)
                            nc.vector.reciprocal_approx_accurate(
                                out=rrec, in_=rep, scratch=rscr)
                            nc.vector.tensor_mul(ctx_t[p][0:64, ic_sl],
                                                 pvt_e[0:64, :], rrec[0:64, :])
                            nc.vector.tensor_mul(ctx_t[p][64:128, ic_sl],
                                                 pvt_o[64:128, :], rrec[64:128, :])

                  # ---- phase 4: out projection (partial), written as out^T
                  # (still inside ps3: po reuses the s_e/s_o tag slots so there
                  # is no psum pool transition barrier)
                  with tc.tile_pool(name="p4", bufs=1) as p4:
                    for et in range(8):
                        for i4 in range(4):
                            k = et * 4 + i4
                            po = ps3.tile([128, 512], F32,
                                          tag=("s_e" if k % 2 == 0 else "s_o"),
                                          bufs=1, name=f"po_{k}")
                            for ct in range(4):
                                nc.tensor.matmul(
                                    out=po, lhsT=wo_s[:, ct, et * 128:(et + 1) * 128],
                                    rhs=ctx_t[ct][:, i4 * 512:(i4 + 1) * 512],
                                    start=(ct == 0), stop=(ct == 3))
                            ot = p4.tile([128, 512], F32, tag="ot", bufs=4)
                            nc.vector.tensor_scalar_add(ot, po, bout_s[:, et, 0:1])
                            nc.sync.dma_start(
                                out=out_d[et * 128:(et + 1) * 128,
                                          i4 * 512:(i4 + 1) * 512],
                                in_=ot)
    nc.compile()
    return nc


_NC = None


def _get_nc():
    global _NC
    if _NC is None:
        _NC = build_nc()
    return _NC


def make_in_maps(query, Wqkv, bqkv, Wout, bout):
    query = np.asarray(query, dtype=np.float32)
    Wqkv = np.asarray(Wqkv, dtype=np.float32)
    bqkv = np.asarray(bqkv, dtype=np.float32)
    Wout = np.asarray(Wout, dtype=np.float32)
    bout = np.asarray(bout, dtype=np.float32)

    in_maps = []
    for c in range(8):
        b, hh = c // 2, c % 2
        heads = np.arange(hh * HL, hh * HL + HL)
        dims = (heads[:, None] * HD + np.arange(HD)[None, :]).reshape(-1)  # [512]
        q_rows, k_rows, v_rows = dims, E + dims, 2 * E + dims

        xw = np.empty((E, 3584), np.float32)
        xw[:, 0:512] = Wqkv[q_rows].T
        xw[:, 512:1024] = Wqkv[k_rows].T
        xw[:, 1024:3072] = query[b].T
        xw[:, 3072:3584] = Wqkv[v_rows].T

        bqk = np.concatenate([bqkv[q_rows], bqkv[k_rows]]).reshape(E, 1)

        cons = np.zeros((1, CONS_LEN), np.float32)
        cons[0, ONES_OFF:ONES_OFF + 128] = 1.0
        cons[0, BV_OFF:BV_OFF + 512] = bqkv[v_rows]
        vpad = np.zeros(V1W, np.float32)
        for i in range(4):
            vpad[i * 65 + 64] = 1.0          # even-head ones col
            vpad[260 + i * 128] = 1.0        # odd-head ones col
        cons[0, VPAD_OFF:VPAD_OFF + V1W] = vpad

        wo = np.ascontiguousarray(Wout[:, dims].T)          # [512, E]
        bo = (bout if hh == 0 else np.zeros_like(bout)).reshape(E, 1)

        in_maps.append({
            "xw": xw, "bqk": np.ascontiguousarray(bqk),
            "cons": cons, "wo": wo, "bout": np.ascontiguousarray(bo),
        })
    return in_maps


def gather(results):
    out = np.empty((B, S, E), np.float32)
    for b in range(B):
        acc = results[2 * b]["outT"] + results[2 * b + 1]["outT"]   # [E, S]
        out[b] = acc.T
    return out


def kernel(query, key, value, Wqkv, bqkv, Wout, bout):
    # key/value are unused by the reference module (qkv all from query)
    nc = _get_nc()
    in_maps = make_in_maps(query, Wqkv, bqkv, Wout, bout)
    res = run_bass_kernel_spmd(nc, in_maps, list(range(8)))
    return gather(res.results)
